# Initial kernel scaffold
#
"""Trainium2 Bass kernel for nn_ConditionDiffusion (reverse-diffusion sampling).

Math (per scan step i = 0..497, derived from the reference):
    t_i   = (499 - i) * DT
    d_i   = b1 + t_i * W1[128, :]              (512,)  hidden preact bias
    c_i   = sqrt(DT) * noise[i] - DT * b2      (128,)  state update bias
    h     = relu(x_i @ W1[:128] + d_i)         (B, 512)
    x_new = A * x_i + h @ (-DT * W2) + c_i     A = 1 - DT/2
    out[:, 499 - i, :] = x_new ;  out[:, 0, :] = x0

Sharding: data-parallel over batch (512 -> 8 cores x 64 rows), weights
replicated.  State kept on-chip in length-major layout xT (128 x 64) so both
matmuls use natural-layout stationary weights; matmuls run in bf16 with fp32
PSUM accumulation and fp32 state; output transposed to batch-major via PE
transpose off the critical path and staged to DRAM in 50-step blocks.
"""

import numpy as np
import ml_dtypes

BATCH, STEPS, LENGTH, HIDDEN = 512, 500, 128, 512
NCORES = 8
ROWS = BATCH // NCORES          # 64 rows per core
NSTEPS = STEPS - 1              # 499
DT = 1e-3
SQDT = float(np.sqrt(np.float32(DT)))
A = 1.0 - 0.5 * DT
NCHUNK = HIDDEN // 128          # 4 hidden chunks
BLK = 50                        # output staging block (time steps)

_PROGRAM_CACHE = {}


def _blocks(nsteps):
    """Output staging blocks: list of (i_start, i_end, k0, width).

    Step i produces time index k = nsteps - i (in 1..nsteps).  Blocks cover
    descending k in chunks of BLK.
    """
    blocks = []
    i = 0
    while i < nsteps:
        width = min(BLK, nsteps - i)
        i_end = i + width
        k0 = nsteps - (i_end - 1)      # smallest k in block
        blocks.append((i, i_end, k0, width))
        i = i_end
    return blocks


def _build_program(nsteps):
    import concourse.bass as bass
    import concourse.mybir as mybir
    from concourse import tile

    f32 = mybir.dt.float32
    bf16 = mybir.dt.bfloat16
    Alu = mybir.AluOpType
    Act = mybir.ActivationFunctionType

    nc = bass.Bass(trn_type="TRN2")

    x0t = nc.declare_dram_parameter("x0t", [LENGTH, ROWS], f32, isOutput=False)
    x0tb = nc.declare_dram_parameter("x0tb", [LENGTH, ROWS], bf16, isOutput=False)
    w1d = nc.declare_dram_parameter("w1", [LENGTH, HIDDEN], bf16, isOutput=False)
    w2d = nc.declare_dram_parameter("w2", [128, HIDDEN], bf16, isOutput=False)
    dd = nc.declare_dram_parameter("d", [128, nsteps * NCHUNK], f32, isOutput=False)
    cd = nc.declare_dram_parameter("c", [128, nsteps], f32, isOutput=False)
    idd = nc.declare_dram_parameter("ident", [128, 128], f32, isOutput=False)
    yd = nc.declare_dram_parameter("y", [ROWS, STEPS, LENGTH], f32, isOutput=True)

    blocks = _blocks(nsteps)

    def block_of(i):
        return i // BLK

    with tile.TileContext(nc) as tc:
        with (
            tc.tile_pool(name="const", bufs=1) as const_pool,
            tc.tile_pool(name="state", bufs=2) as state_pool,
            tc.tile_pool(name="hbuf", bufs=2) as h_pool,
            tc.tile_pool(name="stage", bufs=2) as stg_pool,
            tc.tile_pool(name="ph", bufs=1, space=bass.MemorySpace.PSUM) as ph_pool,
            tc.tile_pool(name="px", bufs=2, space=bass.MemorySpace.PSUM) as px_pool,
            tc.tile_pool(name="po", bufs=2, space=bass.MemorySpace.PSUM) as po_pool,
        ):
            # ---- constants ----
            w1 = const_pool.tile([LENGTH, HIDDEN], bf16, tag="w1")
            w2 = const_pool.tile([128, HIDDEN], bf16, tag="w2")
            dt_ = const_pool.tile([128, nsteps * NCHUNK], f32, tag="d")
            ct = const_pool.tile([128, nsteps], f32, tag="c")
            ident = const_pool.tile([128, 128], f32, tag="ident")
            nc.sync.dma_start(w1[:], w1d[:])
            nc.sync.dma_start(w2[:], w2d[:])
            nc.sync.dma_start(ident[:], idd[:])
            nc.sync.dma_start(ct[:], cd[:])
            nc.sync.dma_start(dt_[:], dd[:])

            # ---- initial state ----
            xf = state_pool.tile([LENGTH, ROWS], f32, tag="xf")
            xb = state_pool.tile([LENGTH, ROWS], bf16, tag="xb")
            nc.sync.dma_start(xf[:], x0t[:])
            nc.sync.dma_start(xb[:], x0tb[:])

            stg_tiles = {}

            def get_stg(b):
                if b not in stg_tiles:
                    _, _, _, width = blocks[b]
                    stg_tiles[b] = stg_pool.tile([ROWS, width, LENGTH], f32, tag="stg")
                return stg_tiles[b]

            # deferred (transpose + output-copy) work for the previous step,
            # emitted inside the current step so it stays off the PE/ACT
            # critical path
            def emit_output_side(i, xf_i1):
                # x_{i+1} (length-major, fp32) -> batch-major staging slot
                po = po_pool.tile([ROWS, LENGTH], f32, tag="po")
                nc.tensor.transpose(po[:], xf_i1[:], ident[:])
                b = block_of(i)
                _, _, k0, _ = blocks[b]
                k = nsteps - i
                stg = get_stg(b)
                nc.scalar.activation(stg[:, k - k0, :], po[:], Act.Copy)
                i0, i_end, k0, width = blocks[b]
                if i == i_end - 1:
                    nc.sync.dma_start(yd[:, k0:k0 + width, :], stg[:])
                    del stg_tiles[b]

            prev_out = None  # (i-1, xf_i) pending output-side work

            for i in range(nsteps):
                # ---- mm1: hT_j = (x @ W1a_j)^T into 4 separate psum banks ----
                phs = []
                for j in range(NCHUNK):
                    ph = ph_pool.tile([128, ROWS], f32, tag=f"ph{j}")
                    nc.tensor.matmul(
                        ph[:], w1[:, j * 128:(j + 1) * 128], xb[:],
                        start=True, stop=True,
                    )
                    phs.append(ph)

                # previous step's transpose fills the PE gap while mm1 psums
                # drain / relus run
                if prev_out is not None:
                    emit_output_side(*prev_out)
                    prev_out = None

                # ---- ax = A * x + c_i  (DVE, off critical path) ----
                ax = state_pool.tile([LENGTH, ROWS], f32, tag="ax")
                nc.vector.tensor_scalar(
                    ax[:], xf[:], A, ct[:, i:i + 1], Alu.mult, Alu.add,
                )

                # ---- relu with per-chunk bias, split ACT/DVE ----
                hs = []
                for j in range(NCHUNK):
                    h = h_pool.tile([128, ROWS], bf16, tag=f"h{j}")
                    dap = dt_[:, i * NCHUNK + j: i * NCHUNK + j + 1]
                    if j % 2 == 0:
                        nc.scalar.activation(h[:], phs[j][:], Act.Relu, bias=dap)
                    else:
                        nc.vector.tensor_scalar(
                            h[:], phs[j][:], dap, 0.0, Alu.add, Alu.max,
                        )
                    hs.append(h)

                # ---- mm2: accumulate x_psum = h @ (-DT*W2) (length-major) ----
                px = px_pool.tile([128, ROWS], f32, tag="px")
                for j in range(NCHUNK):
                    nc.tensor.matmul(
                        px[:], w2[:, j * 128:(j + 1) * 128], hs[j][:],
                        start=(j == 0), stop=(j == NCHUNK - 1),
                    )

                # ---- update: x_new = px + ax (bf16 copy first: critical) ----
                xb_new = state_pool.tile([LENGTH, ROWS], bf16, tag="xb")
                nc.vector.tensor_tensor(xb_new[:], px[:], ax[:], Alu.add)
                xf_new = state_pool.tile([LENGTH, ROWS], f32, tag="xf")
                nc.vector.tensor_tensor(xf_new[:], px[:], ax[:], Alu.add)

                prev_out = (i, xf_new)
                xb, xf = xb_new, xf_new

            # tail: output-side work of the final step
            emit_output_side(*prev_out)

    return nc


def _get_program(nsteps):
    if nsteps not in _PROGRAM_CACHE:
        _PROGRAM_CACHE[nsteps] = _build_program(nsteps)
    return _PROGRAM_CACHE[nsteps]


def _prep_inputs(forward_diffusion, W1, b1, W2, b2, noise, nsteps):
    f32 = np.float32
    bf16 = ml_dtypes.bfloat16

    x0 = np.ascontiguousarray(forward_diffusion[:, -1, :]).astype(f32)  # (B, L)

    t = (np.arange(nsteps, dtype=f32)[::-1] + 1.0) * f32(DT)  # t_i = (nsteps-i)*DT
    w1row = W1[LENGTH, :].astype(f32)
    D = b1[None, :].astype(f32) + t[:, None] * w1row[None, :]        # (nsteps, 512)
    d_in = np.ascontiguousarray(
        D.reshape(nsteps, NCHUNK, 128).transpose(2, 0, 1).reshape(128, nsteps * NCHUNK)
    )
    c_in = np.ascontiguousarray(
        (f32(SQDT) * noise[:nsteps].astype(f32) - f32(DT) * b2[None, :].astype(f32)).T
    )  # (128, nsteps)

    w1_in = np.ascontiguousarray(W1[:LENGTH].astype(bf16))            # (128, 512)
    w2_in = np.ascontiguousarray(
        (-f32(DT) * W2.astype(f32))
        .reshape(NCHUNK, 128, LENGTH).transpose(1, 0, 2).reshape(128, NCHUNK * LENGTH)
        .astype(bf16)
    )
    ident = np.eye(128, dtype=f32)

    shared = {"w1": w1_in, "w2": w2_in, "d": d_in, "c": c_in, "ident": ident}
    in_maps = []
    for cix in range(NCORES):
        x0c = x0[cix * ROWS:(cix + 1) * ROWS]                        # (64, 128)
        x0t_c = np.ascontiguousarray(x0c.T)                          # (128, 64)
        m = dict(shared)
        m["x0t"] = x0t_c
        m["x0tb"] = x0t_c.astype(bf16)
        in_maps.append(m)
    return x0, in_maps


def run_device(inputs, nsteps=NSTEPS, trace=False, tmpdir=None):
    """Build + run the device program; returns (x0, per-core y list, results obj)."""
    from concourse.bass_utils import run_bass_kernel_spmd

    x0, in_maps = _prep_inputs(
        inputs["forward_diffusion"], inputs["W1"], inputs["b1"],
        inputs["W2"], inputs["b2"], inputs["noise"], nsteps,
    )
    nc = _get_program(nsteps)
    res = run_bass_kernel_spmd(
        nc, in_maps, core_ids=list(range(NCORES)), trace=trace, tmpdir=tmpdir,
    )
    return x0, res


def kernel(**inputs):
    x0, res = run_device(inputs)
    out = np.empty((BATCH, STEPS, LENGTH), dtype=np.float32)
    for cix in range(NCORES):
        out[cix * ROWS:(cix + 1) * ROWS] = res.results[cix]["y"]
    out[:, 0, :] = x0
    return out


# revision 8
# speedup vs baseline: 1.0357x; 1.0357x over previous
"""Trainium2 Bass kernel for nn_ConditionDiffusion (reverse-diffusion sampling).

Math (per scan step i = 0..497, derived from the reference):
    t_i   = (499 - i) * DT
    d_i   = b1 + t_i * W1[128, :]              (512,)  hidden preact bias
    c_i   = sqrt(DT) * noise[i] - DT * b2      (128,)  state update bias
    h     = relu(x_i @ W1[:128] + d_i)         (B, 512)
    x_new = A * x_i + h @ (-DT * W2) + c_i     A = 1 - DT/2
    out[:, 499 - i, :] = x_new ;  out[:, 0, :] = x0

Sharding: data-parallel over batch (512 -> 8 cores x 64 rows), weights
replicated.  State kept on-chip in length-major layout xT (128 x 64) so both
matmuls use natural-layout stationary weights; matmuls run in bf16 with fp32
PSUM accumulation and fp32 state; output transposed to batch-major via PE
transpose off the critical path and staged to DRAM in 50-step blocks.
"""

import numpy as np
import ml_dtypes

BATCH, STEPS, LENGTH, HIDDEN = 512, 500, 128, 512
NCORES = 8
ROWS = BATCH // NCORES          # 64 rows per core
NSTEPS = STEPS - 1              # 499
DT = 1e-3
SQDT = float(np.sqrt(np.float32(DT)))
A = 1.0 - 0.5 * DT
NCHUNK = HIDDEN // 128          # 4 hidden chunks
BLK = 50                        # output staging block (time steps)

_PROGRAM_CACHE = {}


def _blocks(nsteps):
    """Output staging blocks: list of (i_start, i_end, k0, width).

    Step i produces time index k = nsteps - i (in 1..nsteps).  Blocks cover
    descending k in chunks of BLK.
    """
    blocks = []
    i = 0
    while i < nsteps:
        width = min(BLK, nsteps - i)
        i_end = i + width
        k0 = nsteps - (i_end - 1)      # smallest k in block
        blocks.append((i, i_end, k0, width))
        i = i_end
    return blocks


def _build_program(nsteps):
    import concourse.bass as bass
    import concourse.bacc as bacc
    import concourse.mybir as mybir
    from concourse import tile

    f32 = mybir.dt.float32
    bf16 = mybir.dt.bfloat16
    Alu = mybir.AluOpType
    Act = mybir.ActivationFunctionType

    nc = bacc.Bacc("TRN2")

    x0t = nc.declare_dram_parameter("x0t", [LENGTH, ROWS], f32, isOutput=False)
    x0tb = nc.declare_dram_parameter("x0tb", [LENGTH, ROWS], bf16, isOutput=False)
    w1d = nc.declare_dram_parameter("w1", [LENGTH, HIDDEN], bf16, isOutput=False)
    w2d = nc.declare_dram_parameter("w2", [128, HIDDEN], bf16, isOutput=False)
    dd = nc.declare_dram_parameter("d", [128, nsteps * NCHUNK], f32, isOutput=False)
    cd = nc.declare_dram_parameter("c", [128, nsteps], f32, isOutput=False)
    idd = nc.declare_dram_parameter("ident", [128, 128], f32, isOutput=False)
    yd = nc.declare_dram_parameter("y", [ROWS, STEPS, LENGTH], f32, isOutput=True)

    blocks = _blocks(nsteps)

    def block_of(i):
        return i // BLK

    with tile.TileContext(nc) as tc:
        with (
            tc.tile_pool(name="const", bufs=1) as const_pool,
            tc.tile_pool(name="state", bufs=2) as state_pool,
            tc.tile_pool(name="hbuf", bufs=2) as h_pool,
            tc.tile_pool(name="stage", bufs=2) as stg_pool,
            tc.tile_pool(name="ph", bufs=1, space=bass.MemorySpace.PSUM) as ph_pool,
            tc.tile_pool(name="px", bufs=2, space=bass.MemorySpace.PSUM) as px_pool,
            tc.tile_pool(name="po", bufs=2, space=bass.MemorySpace.PSUM) as po_pool,
        ):
            # ---- constants ----
            w1 = const_pool.tile([LENGTH, HIDDEN], bf16, tag="w1")
            w2 = const_pool.tile([128, HIDDEN], bf16, tag="w2")
            dt_ = const_pool.tile([128, nsteps * NCHUNK], f32, tag="d")
            ct = const_pool.tile([128, nsteps], f32, tag="c")
            ident = const_pool.tile([128, 128], f32, tag="ident")
            nc.sync.dma_start(w1[:], w1d[:])
            nc.sync.dma_start(w2[:], w2d[:])
            nc.sync.dma_start(ident[:], idd[:])
            nc.sync.dma_start(ct[:], cd[:])
            nc.sync.dma_start(dt_[:], dd[:])

            # ---- initial state ----
            xf = state_pool.tile([LENGTH, ROWS], f32, tag="xf")
            xb = state_pool.tile([LENGTH, ROWS], bf16, tag="xb")
            nc.sync.dma_start(xf[:], x0t[:])
            nc.sync.dma_start(xb[:], x0tb[:])

            # wait-absorbers: make each engine observe the const/init DMA
            # semaphores once, so steady-state ops (esp. TensorScalarPtr,
            # which has few sync-wait slots) carry at most one wait each
            abs_v = const_pool.tile([128, 1], f32, name="abs_v", tag="abs_v")
            nc.vector.tensor_copy(abs_v[:], ct[:, 0:1])
            nc.vector.tensor_copy(abs_v[:], dt_[:, 0:1])
            nc.vector.tensor_copy(abs_v[:], xf[:, 0:1])
            abs_s = const_pool.tile([128, 1], f32, name="abs_s", tag="abs_s")
            nc.scalar.activation(abs_s[:], dt_[:, 0:1], Act.Copy)
            nc.scalar.activation(abs_s[:], ct[:, 0:1], Act.Copy)

            stg_tiles = {}

            def get_stg(b):
                if b not in stg_tiles:
                    _, _, _, width = blocks[b]
                    stg_tiles[b] = stg_pool.tile([ROWS, width, LENGTH], f32, name=f"stg{b}", tag="stg")
                return stg_tiles[b]

            # deferred (transpose + output-copy) work for the previous step,
            # emitted inside the current step so it stays off the PE/ACT
            # critical path
            def emit_output_side(i, xf_i1):
                # x_{i+1} (length-major, fp32) -> batch-major staging slot
                po = po_pool.tile([ROWS, LENGTH], f32, tag="po")
                nc.tensor.transpose(po[:], xf_i1[:], ident[:])
                b = block_of(i)
                _, _, k0, _ = blocks[b]
                k = nsteps - i
                stg = get_stg(b)
                nc.scalar.activation(stg[:, k - k0, :], po[:], Act.Copy)
                i0, i_end, k0, width = blocks[b]
                if i == i_end - 1:
                    nc.sync.dma_start(yd[:, k0:k0 + width, :], stg[:])
                    del stg_tiles[b]

            prev_out = None  # (i-1, xf_i) pending output-side work

            for i in range(nsteps):
                # ---- mm1: hT_j = (x @ W1a_j)^T into 4 separate psum banks ----
                phs = []
                for j in range(NCHUNK):
                    ph = ph_pool.tile([128, ROWS], f32, name=f"ph{j}", tag=f"ph{j}")
                    nc.tensor.matmul(
                        ph[:], w1[:, j * 128:(j + 1) * 128], xb[:],
                        start=True, stop=True,
                    )
                    phs.append(ph)

                # previous step's transpose fills the PE gap while mm1 psums
                # drain / relus run
                if prev_out is not None:
                    emit_output_side(*prev_out)
                    prev_out = None

                # ---- relu with per-chunk bias, split ACT/DVE ----
                hs = []
                for j in range(NCHUNK):
                    h = h_pool.tile([128, ROWS], bf16, name=f"h{j}", tag=f"h{j}")
                    dap = dt_[:, i * NCHUNK + j: i * NCHUNK + j + 1]
                    if j % 2 == 0:
                        nc.scalar.activation(h[:], phs[j][:], Act.Relu, bias=dap)
                    else:
                        nc.vector.tensor_scalar(
                            h[:], phs[j][:], dap, 0.0, Alu.add, Alu.max,
                        )
                    hs.append(h)

                # ---- ax = A * x + c_i  (DVE, off critical path) ----
                ax = state_pool.tile([LENGTH, ROWS], f32, tag="ax")
                nc.vector.tensor_scalar(
                    ax[:], xf[:], A, ct[:, i:i + 1], Alu.mult, Alu.add,
                )

                # ---- mm2: accumulate x_psum = h @ (-DT*W2) (length-major) ----
                px = px_pool.tile([128, ROWS], f32, tag="px")
                for j in range(NCHUNK):
                    nc.tensor.matmul(
                        px[:], w2[:, j * 128:(j + 1) * 128], hs[j][:],
                        start=(j == 0), stop=(j == NCHUNK - 1),
                    )

                # ---- update: x_new = px + ax (bf16 copy first: critical) ----
                xb_new = state_pool.tile([LENGTH, ROWS], bf16, tag="xb")
                nc.vector.tensor_tensor(xb_new[:], px[:], ax[:], Alu.add)
                xf_new = state_pool.tile([LENGTH, ROWS], f32, tag="xf")
                nc.vector.tensor_tensor(xf_new[:], px[:], ax[:], Alu.add)

                prev_out = (i, xf_new)
                xb, xf = xb_new, xf_new

            # tail: output-side work of the final step
            emit_output_side(*prev_out)

    nc.compile()
    return nc


def _get_program(nsteps):
    if nsteps not in _PROGRAM_CACHE:
        _PROGRAM_CACHE[nsteps] = _build_program(nsteps)
    return _PROGRAM_CACHE[nsteps]


def _prep_inputs(forward_diffusion, W1, b1, W2, b2, noise, nsteps):
    f32 = np.float32
    bf16 = ml_dtypes.bfloat16

    x0 = np.ascontiguousarray(forward_diffusion[:, -1, :]).astype(f32)  # (B, L)

    t = (np.arange(nsteps, dtype=f32)[::-1] + 1.0) * f32(DT)  # t_i = (nsteps-i)*DT
    w1row = W1[LENGTH, :].astype(f32)
    D = b1[None, :].astype(f32) + t[:, None] * w1row[None, :]        # (nsteps, 512)
    d_in = np.ascontiguousarray(
        D.reshape(nsteps, NCHUNK, 128).transpose(2, 0, 1).reshape(128, nsteps * NCHUNK)
    )
    c_in = np.ascontiguousarray(
        (f32(SQDT) * noise[:nsteps].astype(f32) - f32(DT) * b2[None, :].astype(f32)).T
    )  # (128, nsteps)

    w1_in = np.ascontiguousarray(W1[:LENGTH].astype(bf16))            # (128, 512)
    w2_in = np.ascontiguousarray(
        (-f32(DT) * W2.astype(f32))
        .reshape(NCHUNK, 128, LENGTH).transpose(1, 0, 2).reshape(128, NCHUNK * LENGTH)
        .astype(bf16)
    )
    ident = np.eye(128, dtype=f32)

    shared = {"w1": w1_in, "w2": w2_in, "d": d_in, "c": c_in, "ident": ident}
    in_maps = []
    for cix in range(NCORES):
        x0c = x0[cix * ROWS:(cix + 1) * ROWS]                        # (64, 128)
        x0t_c = np.ascontiguousarray(x0c.T)                          # (128, 64)
        m = dict(shared)
        m["x0t"] = x0t_c
        m["x0tb"] = x0t_c.astype(bf16)
        in_maps.append(m)
    return x0, in_maps


def run_device(inputs, nsteps=NSTEPS, trace=False, tmpdir=None):
    """Build + run the device program; returns (x0, per-core y list, results obj)."""
    from concourse.bass_utils import run_bass_kernel_spmd

    x0, in_maps = _prep_inputs(
        inputs["forward_diffusion"], inputs["W1"], inputs["b1"],
        inputs["W2"], inputs["b2"], inputs["noise"], nsteps,
    )
    nc = _get_program(nsteps)
    res = run_bass_kernel_spmd(
        nc, in_maps, core_ids=list(range(NCORES)), trace=trace, tmpdir=tmpdir,
    )
    return x0, res


def kernel(**inputs):
    x0, res = run_device(inputs)
    out = np.empty((BATCH, STEPS, LENGTH), dtype=np.float32)
    for cix in range(NCORES):
        out[cix * ROWS:(cix + 1) * ROWS] = res.results[cix]["y"]
    out[:, 0, :] = x0
    return out
